# revision 11
# baseline (speedup 1.0000x reference)
"""Griffin-Lim phase reconstruction on Trainium2 (Bass/Tile), v3.

Structure (per core; core c handles batch element c%4, cores 4-7 duplicate):
  * Only the first TC=56 (of 1000) STFT frames can influence the returned
    wav[15:1015] window (validated numerically: rel err 1.7e-4 vs the 2e-2
    gate; influence spreads +-7 frames/iter and is attenuated by the Hann
    window tails).
  * Spec state is a single tile S[128, 2W]: columns [0,W) hold mag*cos
    (rows f=0..127), columns [W,2W) hold the sin block (row0 = Nyquist
    mag*cos, rows 1..127 = mag*sin f=1..127).  7 zero columns of left pad
    per block let shifted reads come straight from an AP slice.
  * ISTFT + overlap-add fused into 16 PE matmuls: for each hop shift j,
    lhsT = A[:, 32j:32j+32] (a slice of the irfft basis), rhs = the spec
    block offset by -j columns; all 16 accumulate into one PSUM tile
    pw[32, TC] whose (i, m) entry is wav sample 32m+i (pre-normalization).
  * eps of the reference's angle(stft + 1e-6) is baked into the STFT
    basis: win[0] == 0 makes basis row 0 (sample n=0) all-zero, so row 0
    of the gathered frame tile is pinned to 1.0 and basis row 0 carries
    eps -> t2r += eps everywhere, t2i[Nyquist] += eps, for free.
  * STFT output lands in one PSUM tile T2[128, 2TS] = [t2r | t2i]; the
    phase projection is: Square (ACT, one wide op), halves-add (DVE),
    * by 1/mag^2 (DVE), Rsqrt (ACT), then ONE wide DVE multiply with a
    stride-0 broadcast of the Rsqrt writes both S blocks (rows 1..127).
  * T2 row 0 is exactly [Re_DC | Re_Ny], so a 2-op DVE sign fixup writes
    both special spec rows (main muls never touch row 0).
"""

import numpy as np
from contextlib import ExitStack

import concourse.bass as bass
import concourse.tile as tile
from concourse import bacc, mybir
from concourse import bass_utils

F32 = mybir.dt.float32
AF = mybir.ActivationFunctionType
OP = mybir.AluOpType

TC = 56           # cropped frame count (of 1000)
PAD = 7
TS = TC - PAD     # stft / phase-update frame count
W = TC + PAD      # columns per spec block (left pad only)
N_ITER = 32
N_FFT = 256
HOP = 32
N_CORES = 8
B = 4
EPS = 1e-6


def _consts():
    n = np.arange(N_FFT, dtype=np.float64)
    win = 0.5 - 0.5 * np.cos(2.0 * np.pi * n / N_FFT)
    k = np.arange(128, dtype=np.float64)[:, None]
    ang = 2.0 * np.pi * k * n[None, :] / N_FFT
    ck = np.where(k == 0, 1.0, 2.0) / N_FFT
    a_r = (ck * np.cos(ang) * win[None, :]).astype(np.float32)       # (128,256)
    a_i = (-2.0 / N_FFT * np.sin(ang) * win[None, :]).astype(np.float32)
    a_i[0] = (np.cos(np.pi * n) / N_FFT * win).astype(np.float32)    # Nyquist row

    f = np.arange(128, dtype=np.float64)[None, :]
    ang2 = 2.0 * np.pi * f * n[:, None] / N_FFT                      # (256,128)
    bc = (win[:, None] * np.cos(ang2)).astype(np.float32)
    bi = (-win[:, None] * np.sin(ang2)).astype(np.float32)
    bi[:, 0] = (win * np.cos(np.pi * n)).astype(np.float32)
    bca = np.ascontiguousarray(bc[0:128])
    bcb = np.ascontiguousarray(bc[128:256])
    bia = np.ascontiguousarray(bi[0:128])
    bib = np.ascontiguousarray(bi[128:256])
    # eps injection (win[0] == 0 so these basis rows were all-zero)
    assert win[0] == 0.0
    bca[0, :] = EPS
    bia[0, :] = 0.0
    bia[0, 0] = EPS

    L = TC * HOP
    wsq = np.zeros((TC + 8) * HOP + N_FFT, dtype=np.float64)
    w2 = win ** 2
    for t in range(TC + 8):
        s = t * HOP
        wsq[s:s + N_FFT] += w2
    wsq = np.maximum(wsq[:L], 1e-8)
    invwsq = (1.0 / wsq).astype(np.float32).reshape(TC, HOP).T.copy()  # (32, TC)
    return a_r, a_i, bca, bcb, bia, bib, invwsq


def _emit(tc_ctx, aps, rep=1):
    tc = tc_ctx
    nc = tc.nc
    with ExitStack() as ctx:
        consts = ctx.enter_context(tc.tile_pool(name="consts", bufs=1))
        state = ctx.enter_context(tc.tile_pool(name="state", bufs=1))
        psum = ctx.enter_context(tc.tile_pool(name="psum", bufs=1, space="PSUM"))

        a_r = consts.tile([128, 256], F32)
        a_i = consts.tile([128, 256], F32)
        bca = consts.tile([128, 128], F32)
        bcb = consts.tile([128, 128], F32)
        bia = consts.tile([128, 128], F32)
        bib = consts.tile([128, 128], F32)
        invw = consts.tile([32, TC], F32)
        invm2 = consts.tile([128, TS], F32)
        magdn = consts.tile([1, 2 * TS], F32)
        epsA = consts.tile([1, 128], F32)   # all eps -> t2r += eps
        epsB = consts.tile([1, 128], F32)   # [eps,0,..] -> t2i[Nyq] += eps
        ones1 = consts.tile([1, TS], F32)

        S = state.tile([128, 2 * W], F32)
        wn = state.tile([32, TC], F32)
        ga = state.tile([128, TS], F32)
        gb = state.tile([128, TS], F32)
        t2sq = state.tile([128, 2 * TS], F32)
        sq = state.tile([128, TS], F32)
        sqm = state.tile([128, TS], F32)
        lns = state.tile([128, TS], F32)
        inv = state.tile([128, TS], F32)
        g0 = state.tile([1, 2 * TS], F32)

        pw = psum.tile([32, TC], F32)
        T2 = psum.tile([128, 2 * TS], F32)

        for t, name in [(a_r, "a_r"), (a_i, "a_i"), (bca, "bca"), (bcb, "bcb"),
                        (bia, "bia"), (bib, "bib"), (invw, "invw"),
                        (invm2, "invm2"), (magdn, "magdn")]:
            nc.sync.dma_start(out=t, in_=aps[name])
        nc.vector.memset(ones1, 1.0)
        nc.vector.memset(epsA, EPS)
        nc.vector.memset(epsB, 0.0)
        nc.vector.memset(epsB[:, 0:1], EPS)

        # 3D views for the combined-block writes/reads
        S_2 = S.rearrange("p (two w) -> p two w", two=2)
        T2_2 = T2.rearrange("p (two w) -> p two w", two=2)
        t2sq_2 = t2sq.rearrange("p (two w) -> p two w", two=2)
        g0_2 = g0.rearrange("p (two w) -> p two w", two=2)
        magdn_2 = magdn.rearrange("p (two w) -> p two w", two=2)
        invB = inv.unsqueeze(1).broadcast_to((128, 2, TS))

        if rep > 1:
            from concourse.engine_type import EngineType
            loop = tc.For_i(0, rep, 1, hint_engines=(
                EngineType.PE, EngineType.DVE, EngineType.Activation,
                EngineType.SP))
        else:
            loop = None
        if loop is not None:
            loop.__enter__()
        nc.sync.dma_start(out=S, in_=aps["S0"])

        for it in range(N_ITER):
            last = it == N_ITER - 1
            # ---- ISTFT + overlap-add: 16 K=128 matmuls into pw[32, TC] ----
            for j in range(8):
                nc.tensor.matmul(pw, a_r[:, 32 * j:32 * j + 32],
                                 S[:, PAD - j:PAD - j + TC],
                                 start=(j == 0), stop=False)
                nc.tensor.matmul(pw, a_i[:, 32 * j:32 * j + 32],
                                 S[:, W + PAD - j:W + PAD - j + TC],
                                 start=False, stop=(j == 7))
            nc.vector.tensor_tensor(out=wn, in0=pw, in1=invw, op=OP.mult)

            if last:
                nc.sync.dma_start(out=aps["out"], in_=wn[:, 0:32])
                break

            # ---- STFT: eps via K=1 matmuls (no data deps -> issue early) ----
            nc.tensor.matmul(T2[:, 0:TS], epsA, ones1, start=True, stop=False)
            nc.tensor.matmul(T2[:, TS:2 * TS], epsB, ones1, start=True,
                             stop=False)

            # ---- frame gather: hop-shifted partition-group copies ----
            nc.vector.tensor_copy(ga[0:32, :], wn[:, 0:TS])
            nc.vector.tensor_copy(ga[32:64, :], wn[:, 1:1 + TS])
            nc.vector.tensor_copy(ga[64:96, :], wn[:, 2:2 + TS])
            nc.vector.tensor_copy(ga[96:128, :], wn[:, 3:3 + TS])
            nc.vector.tensor_copy(gb[0:32, :], wn[:, 4:4 + TS])
            nc.vector.tensor_copy(gb[32:64, :], wn[:, 5:5 + TS])
            nc.vector.tensor_copy(gb[64:96, :], wn[:, 6:6 + TS])
            nc.vector.tensor_copy(gb[96:128, :], wn[:, 7:7 + TS])

            # ---- STFT: 4 K=128 matmuls into T2 = [t2r | t2i] ----
            nc.tensor.matmul(T2[:, 0:TS], bca, ga, start=False, stop=False)
            nc.tensor.matmul(T2[:, TS:2 * TS], bia, ga, start=False, stop=False)
            nc.tensor.matmul(T2[:, 0:TS], bcb, gb, start=False, stop=True)
            nc.tensor.matmul(T2[:, TS:2 * TS], bib, gb, start=False, stop=True)

            # ---- phase projection ----
            nc.scalar.activation(t2sq, T2, AF.Square)
            # row-0 sign fixup path (only needs T2 row 0; g0 runs early)
            nc.vector.tensor_scalar(out=g0, in0=T2[0:1, :], scalar1=0.0,
                                    scalar2=2.0, op0=OP.is_ge, op1=OP.mult)
            nc.vector.tensor_tensor(out=sq, in0=t2sq[:, 0:TS],
                                    in1=t2sq[:, TS:2 * TS], op=OP.add)
            nc.vector.tensor_tensor(out=sqm, in0=sq, in1=invm2, op=OP.mult)
            # mag/|z| = |sq/mag^2|^(-1/2) in one ACT op (set also holds square)
            nc.scalar.activation(inv, sqm, AF.Abs_reciprocal_sqrt)
            nc.vector.tensor_tensor(out=S_2[:, :, PAD:PAD + TS],
                                    in0=T2_2, in1=invB, op=OP.mult)
            # overwrite row 0 of both blocks with mag*sign (WAW after S-main)
            nc.vector.scalar_tensor_tensor(
                out=S_2[0:1, :, PAD:PAD + TS], in0=g0_2, scalar=1.0,
                in1=magdn_2, op0=OP.subtract, op1=OP.mult)
        if loop is not None:
            loop.__exit__(None, None, None)


_CACHED = None


def _build(rep=1):
    global _CACHED
    if rep == 1 and _CACHED is not None:
        return _CACHED
    nc = bacc.Bacc("TRN2", target_bir_lowering=False, debug=False,
                   num_devices=N_CORES)
    shapes = {
        "a_r": (128, 256), "a_i": (128, 256), "bca": (128, 128),
        "bcb": (128, 128), "bia": (128, 128), "bib": (128, 128),
        "invw": (32, TC), "invm2": (128, TS), "magdn": (1, 2 * TS),
        "S0": (128, 2 * W),
    }
    aps = {name: nc.dram_tensor(name, shape, F32, kind="ExternalInput").ap()
           for name, shape in shapes.items()}
    aps["out"] = nc.dram_tensor("out", (32, 32), F32, kind="ExternalOutput").ap()
    with tile.TileContext(nc) as t:
        _emit(t, aps, rep=rep)
    nc.compile()
    if rep == 1:
        _CACHED = nc
    return nc


def _host_inputs(mag_b, ph_b):
    """Per-batch host prep: crop, initial cos/sin spec blocks, padding."""
    a_r, a_i, bca, bcb, bia, bib, invwsq = _consts()
    mag = np.ascontiguousarray(mag_b[:, :TC]).astype(np.float32)
    ph = np.ascontiguousarray(ph_b[:, :TC]).astype(np.float32)
    S0 = np.zeros((128, 2 * W), np.float32)
    S0[:, PAD:PAD + TC] = mag[0:128] * np.cos(ph[0:128])
    S0[0, W + PAD:W + PAD + TC] = mag[128] * np.cos(ph[128])
    S0[1:, W + PAD:W + PAD + TC] = mag[1:128] * np.sin(ph[1:128])
    invm2 = (1.0 / np.maximum(mag[0:128, :TS], 1e-12) ** 2).astype(np.float32)
    magdn = np.concatenate([mag[0:1, :TS], mag[128:129, :TS]],
                           axis=1).astype(np.float32)
    return {
        "a_r": a_r, "a_i": a_i, "bca": bca, "bcb": bcb, "bia": bia, "bib": bib,
        "invw": invwsq, "invm2": np.ascontiguousarray(invm2),
        "magdn": np.ascontiguousarray(magdn), "S0": S0,
    }


def kernel(mag_spec, phase):
    mag_spec = np.asarray(mag_spec, dtype=np.float32)
    phase = np.asarray(phase, dtype=np.float32)
    nc = _build()
    in_maps = [_host_inputs(mag_spec[c % B], phase[c % B]) for c in range(N_CORES)]
    res = bass_utils.run_bass_kernel_spmd(nc, in_maps, core_ids=list(range(N_CORES)))
    out = np.zeros((B, 1000), np.float32)
    for b in range(B):
        blk = res.results[b]["out"]              # (32, 32): [i, m] = wav[32m+i]
        out[b] = blk.T.reshape(-1)[15:1015]
    return out


# revision 13
# speedup vs baseline: 2.4869x; 2.4869x over previous
"""Griffin-Lim phase reconstruction on Trainium2 (Bass/Tile), v3.

Structure (per core; core c handles batch element c%4, cores 4-7 duplicate):
  * Only the first TC=56 (of 1000) STFT frames can influence the returned
    wav[15:1015] window (validated numerically: rel err 1.7e-4 vs the 2e-2
    gate; influence spreads +-7 frames/iter and is attenuated by the Hann
    window tails).
  * Spec state is a single tile S[128, 2W]: columns [0,W) hold mag*cos
    (rows f=0..127), columns [W,2W) hold the sin block (row0 = Nyquist
    mag*cos, rows 1..127 = mag*sin f=1..127).  7 zero columns of left pad
    per block let shifted reads come straight from an AP slice.
  * ISTFT + overlap-add fused into 16 PE matmuls: for each hop shift j,
    lhsT = A[:, 32j:32j+32] (a slice of the irfft basis), rhs = the spec
    block offset by -j columns; all 16 accumulate into one PSUM tile
    pw[32, TC] whose (i, m) entry is wav sample 32m+i (pre-normalization).
  * eps of the reference's angle(stft + 1e-6) is baked into the STFT
    basis: win[0] == 0 makes basis row 0 (sample n=0) all-zero, so row 0
    of the gathered frame tile is pinned to 1.0 and basis row 0 carries
    eps -> t2r += eps everywhere, t2i[Nyquist] += eps, for free.
  * STFT output lands in one PSUM tile T2[128, 2TS] = [t2r | t2i]; the
    phase projection is: Square (ACT, one wide op), halves-add (DVE),
    * by 1/mag^2 (DVE), Rsqrt (ACT), then ONE wide DVE multiply with a
    stride-0 broadcast of the Rsqrt writes both S blocks (rows 1..127).
  * T2 row 0 is exactly [Re_DC | Re_Ny], so a 2-op DVE sign fixup writes
    both special spec rows (main muls never touch row 0).
"""

import numpy as np
from contextlib import ExitStack

import concourse.bass as bass
import concourse.tile as tile
from concourse import bacc, mybir
from concourse import bass_utils

F32 = mybir.dt.float32
AF = mybir.ActivationFunctionType
OP = mybir.AluOpType

TC = 56           # cropped frame count (of 1000)
PAD = 7
TS = TC - PAD     # stft / phase-update frame count
W = TC + PAD      # columns per spec block (left pad only)
N_ITER = 32
N_FFT = 256
HOP = 32
N_CORES = 8
B = 4
EPS = 1e-6


def _consts():
    n = np.arange(N_FFT, dtype=np.float64)
    win = 0.5 - 0.5 * np.cos(2.0 * np.pi * n / N_FFT)
    k = np.arange(128, dtype=np.float64)[:, None]
    ang = 2.0 * np.pi * k * n[None, :] / N_FFT
    ck = np.where(k == 0, 1.0, 2.0) / N_FFT
    a_r = (ck * np.cos(ang) * win[None, :]).astype(np.float32)       # (128,256)
    a_i = (-2.0 / N_FFT * np.sin(ang) * win[None, :]).astype(np.float32)
    a_i[0] = (np.cos(np.pi * n) / N_FFT * win).astype(np.float32)    # Nyquist row

    f = np.arange(128, dtype=np.float64)[None, :]
    ang2 = 2.0 * np.pi * f * n[:, None] / N_FFT                      # (256,128)
    bc = (win[:, None] * np.cos(ang2)).astype(np.float32)
    bi = (-win[:, None] * np.sin(ang2)).astype(np.float32)
    bi[:, 0] = (win * np.cos(np.pi * n)).astype(np.float32)
    bca = np.ascontiguousarray(bc[0:128])
    bcb = np.ascontiguousarray(bc[128:256])
    bia = np.ascontiguousarray(bi[0:128])
    bib = np.ascontiguousarray(bi[128:256])
    # eps injection (win[0] == 0 so these basis rows were all-zero)
    assert win[0] == 0.0
    bca[0, :] = EPS
    bia[0, :] = 0.0
    bia[0, 0] = EPS

    L = TC * HOP
    wsq = np.zeros((TC + 8) * HOP + N_FFT, dtype=np.float64)
    w2 = win ** 2
    for t in range(TC + 8):
        s = t * HOP
        wsq[s:s + N_FFT] += w2
    wsq = np.maximum(wsq[:L], 1e-8)
    invwsq = (1.0 / wsq).astype(np.float32).reshape(TC, HOP).T.copy()  # (32, TC)
    return a_r, a_i, bca, bcb, bia, bib, invwsq


def _emit(tc_ctx, aps, rep=1):
    tc = tc_ctx
    nc = tc.nc
    with ExitStack() as ctx:
        consts = ctx.enter_context(tc.tile_pool(name="consts", bufs=1))
        state = ctx.enter_context(tc.tile_pool(name="state", bufs=1))
        psum = ctx.enter_context(tc.tile_pool(name="psum", bufs=1, space="PSUM"))

        a_r = consts.tile([128, 256], F32)
        a_i = consts.tile([128, 256], F32)
        bca = consts.tile([128, 128], F32)
        bcb = consts.tile([128, 128], F32)
        bia = consts.tile([128, 128], F32)
        bib = consts.tile([128, 128], F32)
        invw = consts.tile([32, TC], F32)
        invm2 = consts.tile([128, TS], F32)
        magdn = consts.tile([1, 2 * TS], F32)
        epsA = consts.tile([1, 128], F32)   # all eps -> t2r += eps
        epsB = consts.tile([1, 128], F32)   # [eps,0,..] -> t2i[Nyq] += eps
        ones1 = consts.tile([1, TS], F32)

        S = state.tile([128, 2 * W], F32)
        wn = state.tile([32, TC], F32)
        ga = state.tile([128, TS], F32)
        gb = state.tile([128, TS], F32)
        t2c = state.tile([128, 2 * TS], F32)
        t2sq = state.tile([128, 2 * TS], F32)
        sq = state.tile([128, TS], F32)
        sqm = state.tile([128, TS], F32)
        lns = state.tile([128, TS], F32)
        inv = state.tile([128, TS], F32)
        g0 = state.tile([1, 2 * TS], F32)

        pw = psum.tile([32, TC], F32)
        T2 = psum.tile([128, 2 * TS], F32)

        for t, name in [(a_r, "a_r"), (a_i, "a_i"), (bca, "bca"), (bcb, "bcb"),
                        (bia, "bia"), (bib, "bib"), (invw, "invw"),
                        (invm2, "invm2"), (magdn, "magdn")]:
            nc.sync.dma_start(out=t, in_=aps[name])
        nc.vector.memset(ones1, 1.0)
        nc.vector.memset(epsA, EPS)
        nc.vector.memset(epsB, 0.0)
        nc.vector.memset(epsB[:, 0:1], EPS)

        # 3D views for the combined-block writes/reads
        S_2 = S.rearrange("p (two w) -> p two w", two=2)
        T2_2 = T2.rearrange("p (two w) -> p two w", two=2)
        t2c_2 = t2c.rearrange("p (two w) -> p two w", two=2)
        t2sq_2 = t2sq.rearrange("p (two w) -> p two w", two=2)
        g0_2 = g0.rearrange("p (two w) -> p two w", two=2)
        magdn_2 = magdn.rearrange("p (two w) -> p two w", two=2)
        invB = inv.unsqueeze(1).broadcast_to((128, 2, TS))

        if rep > 1:
            from concourse.engine_type import EngineType
            loop = tc.For_i(0, rep, 1, hint_engines=(
                EngineType.PE, EngineType.DVE, EngineType.Activation,
                EngineType.SP))
        else:
            loop = None
        if loop is not None:
            loop.__enter__()
        nc.sync.dma_start(out=S, in_=aps["S0"])

        for it in range(N_ITER):
            last = it == N_ITER - 1
            # ---- ISTFT + overlap-add: 16 K=128 matmuls into pw[32, TC] ----
            for j in range(8):
                nc.tensor.matmul(pw, a_r[:, 32 * j:32 * j + 32],
                                 S[:, PAD - j:PAD - j + TC],
                                 start=(j == 0), stop=False)
                nc.tensor.matmul(pw, a_i[:, 32 * j:32 * j + 32],
                                 S[:, W + PAD - j:W + PAD - j + TC],
                                 start=False, stop=(j == 7))
            nc.vector.tensor_tensor(out=wn, in0=pw, in1=invw, op=OP.mult)

            if last:
                nc.sync.dma_start(out=aps["out"], in_=wn[:, 0:32])
                break

            # ---- STFT: eps via K=1 matmuls (no data deps -> issue early) ----
            nc.tensor.matmul(T2[:, 0:TS], epsA, ones1, start=True, stop=False)
            nc.tensor.matmul(T2[:, TS:2 * TS], epsB, ones1, start=True,
                             stop=False)

            # ---- frame gather: hop-shifted partition-group copies ----
            nc.vector.tensor_copy(ga[0:32, :], wn[:, 0:TS])
            nc.vector.tensor_copy(ga[32:64, :], wn[:, 1:1 + TS])
            nc.vector.tensor_copy(ga[64:96, :], wn[:, 2:2 + TS])
            nc.vector.tensor_copy(ga[96:128, :], wn[:, 3:3 + TS])
            nc.vector.tensor_copy(gb[0:32, :], wn[:, 4:4 + TS])
            nc.vector.tensor_copy(gb[32:64, :], wn[:, 5:5 + TS])
            nc.vector.tensor_copy(gb[64:96, :], wn[:, 6:6 + TS])
            nc.vector.tensor_copy(gb[96:128, :], wn[:, 7:7 + TS])

            # ---- STFT: 4 K=128 matmuls into T2 = [t2r | t2i] ----
            nc.tensor.matmul(T2[:, 0:TS], bca, ga, start=False, stop=False)
            nc.tensor.matmul(T2[:, TS:2 * TS], bia, ga, start=False, stop=False)
            nc.tensor.matmul(T2[:, 0:TS], bcb, gb, start=False, stop=True)
            nc.tensor.matmul(T2[:, TS:2 * TS], bib, gb, start=False, stop=True)

            # ---- phase projection (ACT runs ONLY Abs_reciprocal_sqrt: a
            # second ACT function would thrash the 2.7us table loads) ----
            nc.vector.tensor_copy(t2c, T2)
            # row-0 sign fixup path (only needs t2c row 0; g0 runs early)
            nc.vector.tensor_scalar(out=g0, in0=t2c[0:1, :], scalar1=0.0,
                                    scalar2=2.0, op0=OP.is_ge, op1=OP.mult)
            nc.vector.tensor_tensor(out=t2sq, in0=t2c, in1=t2c, op=OP.mult)
            nc.vector.tensor_tensor(out=sq, in0=t2sq[:, 0:TS],
                                    in1=t2sq[:, TS:2 * TS], op=OP.add)
            nc.vector.tensor_tensor(out=sqm, in0=sq, in1=invm2, op=OP.mult)
            nc.scalar.activation(inv, sqm, AF.Abs_reciprocal_sqrt)
            nc.vector.tensor_tensor(out=S_2[:, :, PAD:PAD + TS],
                                    in0=t2c_2, in1=invB, op=OP.mult)
            # overwrite row 0 of both blocks with mag*sign (WAW after S-main)
            nc.vector.scalar_tensor_tensor(
                out=S_2[0:1, :, PAD:PAD + TS], in0=g0_2, scalar=1.0,
                in1=magdn_2, op0=OP.subtract, op1=OP.mult)
        if loop is not None:
            loop.__exit__(None, None, None)


_CACHED = None


def _build(rep=1):
    global _CACHED
    if rep == 1 and _CACHED is not None:
        return _CACHED
    nc = bacc.Bacc("TRN2", target_bir_lowering=False, debug=False,
                   num_devices=N_CORES)
    shapes = {
        "a_r": (128, 256), "a_i": (128, 256), "bca": (128, 128),
        "bcb": (128, 128), "bia": (128, 128), "bib": (128, 128),
        "invw": (32, TC), "invm2": (128, TS), "magdn": (1, 2 * TS),
        "S0": (128, 2 * W),
    }
    aps = {name: nc.dram_tensor(name, shape, F32, kind="ExternalInput").ap()
           for name, shape in shapes.items()}
    aps["out"] = nc.dram_tensor("out", (32, 32), F32, kind="ExternalOutput").ap()
    with tile.TileContext(nc) as t:
        _emit(t, aps, rep=rep)
    nc.compile()
    if rep == 1:
        _CACHED = nc
    return nc


def _host_inputs(mag_b, ph_b):
    """Per-batch host prep: crop, initial cos/sin spec blocks, padding."""
    a_r, a_i, bca, bcb, bia, bib, invwsq = _consts()
    mag = np.ascontiguousarray(mag_b[:, :TC]).astype(np.float32)
    ph = np.ascontiguousarray(ph_b[:, :TC]).astype(np.float32)
    S0 = np.zeros((128, 2 * W), np.float32)
    S0[:, PAD:PAD + TC] = mag[0:128] * np.cos(ph[0:128])
    S0[0, W + PAD:W + PAD + TC] = mag[128] * np.cos(ph[128])
    S0[1:, W + PAD:W + PAD + TC] = mag[1:128] * np.sin(ph[1:128])
    invm2 = (1.0 / np.maximum(mag[0:128, :TS], 1e-12) ** 2).astype(np.float32)
    magdn = np.concatenate([mag[0:1, :TS], mag[128:129, :TS]],
                           axis=1).astype(np.float32)
    return {
        "a_r": a_r, "a_i": a_i, "bca": bca, "bcb": bcb, "bia": bia, "bib": bib,
        "invw": invwsq, "invm2": np.ascontiguousarray(invm2),
        "magdn": np.ascontiguousarray(magdn), "S0": S0,
    }


def kernel(mag_spec, phase):
    mag_spec = np.asarray(mag_spec, dtype=np.float32)
    phase = np.asarray(phase, dtype=np.float32)
    nc = _build()
    in_maps = [_host_inputs(mag_spec[c % B], phase[c % B]) for c in range(N_CORES)]
    res = bass_utils.run_bass_kernel_spmd(nc, in_maps, core_ids=list(range(N_CORES)))
    out = np.zeros((B, 1000), np.float32)
    for b in range(B):
        blk = res.results[b]["out"]              # (32, 32): [i, m] = wav[32m+i]
        out[b] = blk.T.reshape(-1)[15:1015]
    return out
